# revision 16
# baseline (speedup 1.0000x reference)
"""Trainium2 Bass kernel for nn_Attention_71966472012100.

Multi-head attention layer (dense_transformer), B=4, S=2048, H=12, D=100,
HID=1200, with the reference's bug-faithful head-mixing reshape before the
output projection.

Sharding: 8 cores = data-parallel over batch (4) x tensor-parallel over head
groups (2 groups of 6 heads). Because the reference reshapes (b, h, s, d) ->
(b, s, h*d) WITHOUT permuting heads back, head-group g's attention output
occupies exactly rows [g*1024, (g+1)*1024) of the reshaped activation.
Each core produces 1024 complete rows of the final output; no cross-core
communication.

v2 (all-f32r): HW measurement shows 16-bit matmuls pay ~240ns per weight
load (stationary changes every matmul here), while f32r weight loads are
free (~215ns for a 512-col matmul = pure streaming). So EVERY matmul in
this kernel is f32r x f32r (also strictly better numerics than the old
bf16 PV/WO path). Key layouts:
  QT/KT: [d, s] per head f32r
  scores^T: [t, s] in PSUM (softmax axis t on partitions; alibi+mask are a
         per-partition bias fused into the ACT exp)
  pt = exp(scores) [t, s] f32r in SBUF (ACT writes f32r directly)
  V': [t, d|ones] f32r (ones column makes PV also produce softmax sums)
  O^T blocks: normalized [d, 512] f32r staged to a DRAM scratch `otd`
         (SBUF cannot hold the full f32 O^T); WO streams it back per
         128-row output block ([d, 1536] chunks).

Schedule (emission order = priority for the dependency-driven scheduler):
  - x^T streams in s-half-major chunk order; V' t-tile groups and the
    first two heads' Q/K s-block projections consume chunks as they land,
    so the PE is busy through the whole x DMA window.
  - attention per head: QK -> exp(ACT, f32r out) -> PV into per-512 PSUM;
    evacuation stages O^T blocks + sums row, deferred-normalizes (recip +
    rank-1 ones broadcast + in-place mul) and DMAs blocks to otd.
  - remaining heads' Q/K proj emitted inside the attention loop (2-deep
    rotating qt/kt buffers) and absorbed into the windows' ACT slack.
  - WO emitted last (lowest priority): per 128-row block, reload the
    [d, 1536] otd chunk, accumulate 12 m-chunks x 3 col-pieces in PSUM,
    stream y out. Runs during the tail windows' bubbles + after.
PSUM (8 banks): startup V'-group 6 + transient 2; attention ss 2x2 +
po 2x1 + transient 2; WO reuses the 2 transient banks for psy.
"""

import numpy as np
from contextlib import ExitStack

import concourse.bass as bass
import concourse.tile as tile
from concourse import bacc
from concourse import mybir
from concourse.bass_utils import run_bass_kernel_spmd

F32 = mybir.dt.float32
F32R = mybir.dt.float32r
BF16 = mybir.dt.bfloat16
EXP = mybir.ActivationFunctionType.Exp

B, S, H, D, HID = 4, 2048, 12, 100, 1200
HG = 2                # head groups (tensor parallel)
HL = H // HG          # 6 heads per core
ROWS = S * HL * D // HID   # 1024 output rows per core
CK, CCH = 120, 10     # contraction chunking of HID
TT = S // 128         # 16 key tiles
VW = HL * (D + 1)     # 606: V' row width per t-tile (d cols + ones col per head)
NM = HID // D         # 12 output-projection contraction chunks
RT = ROWS // 128      # 8 output row blocks
RSPAN = 128 * NM      # 1536 O^T columns per output row block


def _mm(nc, out, lhsT, rhs, **kw):
    nc.tensor.matmul(out, lhsT, rhs, **kw)


def _absorb(nc, ap):
    """PE-side observation of a freshly DMA'd tile (absorbs a DMA wait in a
    throwaway 1-column LDWEIGHTS ahead of the real matmuls)."""
    bb = ap.bitcast(BF16)
    nc.tensor.ldweights(bb[:, 0:1])


def build_program(scale: float, debug_taps: bool = False, n_iters: int = 1):
    nc = bacc.Bacc("TRN2", target_bir_lowering=False, debug=False)

    tn = {}
    tn["xT"] = nc.dram_tensor("xT", [HID, S], F32R, kind="ExternalInput")
    # per-head packed q||k weights: [120, c(10) * 200] f32r
    tn["wqk"] = nc.dram_tensor("wqk", [HL, CK, CCH * 2 * D], F32R,
                               kind="ExternalInput")
    # packed v weights: [120, c(10) * 600] f32r
    tn["wv"] = nc.dram_tensor("wv", [CK, CCH * HL * D], F32R,
                              kind="ExternalInput")
    # packed wo: [100, m(12) * 1200] f32r
    tn["wo"] = nc.dram_tensor("wo", [D, NM * HID], F32R, kind="ExternalInput")
    tn["biasT"] = nc.dram_tensor("biasT", [128, HL * TT], F32,
                                 kind="ExternalInput")
    # DRAM scratch for the normalized O^T (too big for SBUF at f32)
    tn["otd"] = nc.dram_tensor("otd", [D, HL * S], F32R, kind="Internal")
    tn["y"] = nc.dram_tensor("y", [ROWS, HID], F32, kind="ExternalOutput")

    with tile.TileContext(nc) as tc:
        for _ in range(n_iters):
            _emit_iter(nc, tc, tn, scale)
    nc.compile()
    return nc


def _emit_iter(nc, tc, tn, scale):
    xT, wqk, wv, wo, biasT, otd, y = (
        tn["xT"], tn["wqk"], tn["wv"], tn["wo"], tn["biasT"], tn["otd"],
        tn["y"])

    with ExitStack() as ctx:
        pa = ctx.enter_context(tc.tile_pool(name="pa", bufs=1))
        vp = pa.tile([128, TT * VW], F32R, name="vp")
        bias_sb = pa.tile([128, HL * TT], F32, name="bias_sb")
        ones1 = pa.tile([1, D], F32R, name="ones1")
        pqk = ctx.enter_context(tc.tile_pool(name="pqk", bufs=2))
        # bufs=2: head h+1's weights DMA must not WAR-wait on head h's
        # still-pending later-emitted proj reads (PE streams are in-order,
        # so that wait would be a head-of-line deadlock).
        pwqk = ctx.enter_context(tc.tile_pool(name="pwqk", bufs=2))

        qk_tiles = {}
        w_tiles = {}

        def emit_wqk_dma(h):
            w_sb = pwqk.tile([CK, CCH * 2 * D], F32R, tag="wqk", name="w_sb")
            nc.sync.dma_start(out=w_sb, in_=wqk.ap()[h])
            _absorb(nc, w_sb)
            w_tiles[h] = w_sb

        def emit_proj(h, sbs, last):
            """Q/K projection of head h, s-blocks `sbs`, from resident x^T
            into rotating [D, S] f32r tiles (transient PSUM pool)."""
            w_sb = w_tiles[h]
            if h in qk_tiles:
                qt, kt = qk_tiles[h]
            else:
                qt = pqk.tile([D, S], F32R, tag="qt", name="qt")
                kt = pqk.tile([D, S], F32R, tag="kt", name="kt")
                qk_tiles[h] = (qt, kt)
            for sb in sbs:
                for qk, dest in ((0, qt), (1, kt)):
                    acc = ptr.tile([D, 512], F32, tag="tr", name="acc")
                    for c in range(CCH):
                        _mm(nc, acc[:, :],
                            w_sb[:, c * 2 * D + qk * D: c * 2 * D + (qk + 1) * D],
                            xt[:, c * S + sb * 512: c * S + (sb + 1) * 512],
                            start=(c == 0), stop=(c == CCH - 1))
                    nc.vector.tensor_copy(
                        out=dest[:, sb * 512:(sb + 1) * 512], in_=acc[:, :])
            if last:
                w_tiles.pop(h)

        def emit_vprime(tts):
            """V' for t-tiles `tts` (<=3 at a time), both jh head-halves,
            all f32r. accs: 2*len(tts) banks + 2 transient <= 8."""
            with tc.tile_pool(name=f"psv{tts[0]}", bufs=2 * len(tts),
                              space="PSUM") as psv:
                accs = {(jh, t): psv.tile([128, 3 * D], F32, tag="vacc",
                                          name="vacc")
                        for jh in range(2) for t in tts}
                for c in range(CCH):
                    for jh in range(2):
                        for t in tts:
                            _mm(nc, accs[(jh, t)][:, :],
                                xt[:, c * S + t * 128: c * S + t * 128 + 128],
                                wv_sb[:, c * HL * D + jh * 3 * D:
                                      c * HL * D + (jh + 1) * 3 * D],
                                start=(c == 0), stop=(c == CCH - 1))
                for (jh, t), acc in accs.items():
                    dst = vp[:, t * VW + jh * 3 * (D + 1):
                             t * VW + (jh + 1) * 3 * (D + 1)]
                    dst3 = dst.rearrange("p (h e) -> p h e", e=D + 1)
                    nc.vector.tensor_copy(out=dst3[:, :, 0:D],
                                          in_=acc.rearrange(
                                              "p (h d) -> p h d", d=D))

        pending_norm = []

        def emit_norm_finish():
            """recip + rank-1 broadcast + in-place multiply + DMA-out for a
            previously evacuated (h, sh) block pair. Deferred one s-half
            window so the pb matmuls sit behind the next tt-loop in the
            static PE order (they depend on the slow DVE chain)."""
            if not pending_norm:
                return
            for stg, h, col in pending_norm.pop(0):
                srow1 = pnr.tile([1, 1024], F32R, tag="srow1", name="srow1",
                                 bufs=1)
                nc.sync.dma_start(out=srow1, in_=stg[D:D + 1, :])
                rrow = pnr.tile([1, 1024], F32R, tag="rrow", name="rrow",
                                bufs=1)
                with nc.allow_low_precision(reason="f32r recip of sums"):
                    nc.vector.reciprocal(out=rrow, in_=srow1)
                for sbb in range(2):
                    pb = ptr.tile([D, 512], F32, tag="tr", name="pb")
                    _mm(nc, pb[:, :], ones1[0:1, :],
                        rrow[0:1, sbb * 512:(sbb + 1) * 512],
                        start=True, stop=True)
                    nc.vector.tensor_mul(
                        stg[0:D, sbb * 512:(sbb + 1) * 512],
                        stg[0:D, sbb * 512:(sbb + 1) * 512], pb[:, :])
                nc.sync.dma_start(out=otd.ap()[:, h * S + col:
                                               h * S + col + 1024],
                                  in_=stg[0:D, :])

        def emit_attention_sh(h, sh, qt, kt):
            s0 = sh * 1024
            pos = [ppo.tile([D + 1, 512], F32, tag="po", name="po")
                   for _ in range(2)]
            for tt in range(TT):
                ss = pss.tile([128, 1024], F32, tag="ss", name="ss")
                for sbb in range(2):
                    _mm(nc, ss[:, sbb * 512:(sbb + 1) * 512],
                        kt[:, tt * 128:(tt + 1) * 128],
                        qt[:, s0 + sbb * 512: s0 + (sbb + 1) * 512],
                        start=True, stop=True)
                pt = ppt.tile([128, 1024], F32R, tag="pt", name="pt")
                nc.scalar.activation(
                    out=pt, in_=ss[:, :], func=EXP,
                    bias=bias_sb[:, h * TT + tt: h * TT + tt + 1],
                    scale=scale)
                for sbb in range(2):
                    _mm(nc, pos[sbb][:, :],
                        vp[:, tt * VW + h * (D + 1):
                           tt * VW + (h + 1) * (D + 1)],
                        pt[:, sbb * 512:(sbb + 1) * 512],
                        start=(tt == 0), stop=(tt == TT - 1))
            # evacuation: ONE [d+1, 512] copy per block (O^T rows + the sums
            # row together) frees each po bank promptly; both blocks land in
            # a single [d+1, 1024] stage so the hop/recip/otd-DMA of the
            # whole s-half is one instruction each.
            stg = pst.tile([D + 1, 1024], F32R, tag="st", name="stg")
            for sbb in range(2):
                nc.vector.tensor_copy(out=stg[:, sbb * 512:(sbb + 1) * 512],
                                      in_=pos[sbb][0:D + 1, :])
            pending_norm.append([(stg, h, s0)])

        # ================= emission (priority) order =================
        nc.sync.dma_start(out=bias_sb, in_=biasT.ap())

        # shared 2-bank transient PSUM: proj accs, norm pb, and WO psy.
        # Opened OUTSIDE pxt (PSUM/SBUF pool stacks are independent) so the
        # WO block can allocate its accumulators from the already-open pool
        # instead of waiting for the attention PSUM pools to drain.
        with tc.tile_pool(name="ptr", bufs=2, space="PSUM") as ptr:
            with tc.tile_pool(name="pxt", bufs=1) as pxt:
                xt = pxt.tile([CK, CCH * S], F32R, name="xt")
                with tc.tile_pool(name="pwv", bufs=1) as pwv:
                    wv_sb = pwv.tile([CK, CCH * HL * D], F32R, name="wv_sb")
                    nc.vector.memset(ones1.bitcast(F32), 1.0)
                    nc.vector.tensor_copy(out=ones1, in_=ones1.bitcast(F32))
                    # ones cols pre-set; V cols overwritten by V' copies.
                    nc.vector.memset(vp.bitcast(F32), 1.0)
                    nc.vector.tensor_copy(out=vp, in_=vp.bitcast(F32))

                    # x streams in s-half-major order, with wv's c-chunks
                    # riding along the first half, so the V' c-loops (and
                    # later the first heads' s-block projections) start
                    # consuming after the very first chunk pair lands.
                    def x_chunk(half, c):
                        nc.sync.dma_start(
                            out=xt[:, c * S + half * 1024:
                                   c * S + (half + 1) * 1024],
                            in_=xT.ap()[c * CK:(c + 1) * CK,
                                        half * 1024:(half + 1) * 1024])
                        _absorb(nc, xt[:, c * S + half * 1024:
                                       c * S + half * 1024 + 1024])

                    for c in range(CCH):
                        nc.sync.dma_start(
                            out=wv_sb[:, c * HL * D:(c + 1) * HL * D],
                            in_=wv.ap()[:, c * HL * D:(c + 1) * HL * D])
                        x_chunk(0, c)
                    emit_wqk_dma(0)
                    emit_wqk_dma(1)
                    for c in range(CCH):
                        x_chunk(1, c)

                    # consume x chunks as they arrive
                    emit_vprime(range(0, 3))
                    emit_vprime(range(3, 6))
                    emit_proj(0, (0, 1), last=False)
                    emit_vprime(range(6, 8))
                    emit_proj(1, (0, 1), last=False)
                    emit_vprime(range(8, 11))
                    emit_proj(0, (2, 3), last=True)
                    emit_vprime(range(11, 14))
                    emit_proj(1, (2, 3), last=True)
                    emit_vprime(range(14, 16))

                with tc.tile_pool(name="pst2", bufs=4) as pst, \
                     tc.tile_pool(name="ppt2", bufs=3) as ppt, \
                     tc.tile_pool(name="pnr2", bufs=2) as pnr, \
                     tc.tile_pool(name="pss", bufs=2, space="PSUM") as pss, \
                     tc.tile_pool(name="ppo", bufs=2, space="PSUM") as ppo:
                    for h in range(6):
                        if h >= 2:
                            emit_wqk_dma(h)
                            emit_proj(h, (0, 1, 2, 3), last=True)
                        qt, kt = qk_tiles.pop(h)
                        for sh in range(2):
                            emit_attention_sh(h, sh, qt, kt)
                            if len(pending_norm) > 1:
                                emit_norm_finish()
                    emit_norm_finish()
                    emit_norm_finish()

            # ==== output projection: emitted last (lowest priority) so it
            # fills the tail windows' bubbles. wop lands in xt's freed SBUF
            # (alloc waits on pxt release = last proj read). O^T streams
            # back from DRAM per 128-row block; psy reuses the open ptr
            # transient banks so nothing waits on attention-PSUM drain. ====
            with tc.tile_pool(name="pwo", bufs=1) as pwo, \
                 tc.tile_pool(name="potc", bufs=3) as potc, \
                 tc.tile_pool(name="pyb", bufs=3) as pyb:
                wop = pwo.tile([D, NM * HID], F32R, name="wop")
                for pc in range(3):
                    nc.sync.dma_start(
                        out=wop[:, pc * 4 * HID:(pc + 1) * 4 * HID],
                        in_=wo.ap()[:, pc * 4 * HID:(pc + 1) * 4 * HID])
                    _absorb(nc, wop[:, pc * 4 * HID:(pc + 1) * 4 * HID])
                for rt in range(RT):
                    otc = potc.tile([D, RSPAN], F32R, tag="otc", name="otc")
                    nc.sync.dma_start(
                        out=otc,
                        in_=otd.ap()[:, rt * RSPAN:(rt + 1) * RSPAN])
                    _absorb(nc, otc)
                    otc3 = otc.rearrange("p (r m) -> p r m", m=NM)
                    ysb = pyb.tile([128, HID], F32, tag="ysb", name="ysb")
                    for jb in range(3):
                        psy = ptr.tile([128, 512], F32, tag="tr",
                                       name="psy")
                        for m in range(NM):
                            _mm(nc, psy[:, 0:400],
                                otc3[:, :, m],
                                wop[:, m * HID + jb * 400:
                                    m * HID + (jb + 1) * 400],
                                start=(m == 0), stop=(m == NM - 1))
                        nc.any.tensor_copy(
                            out=ysb[:, jb * 400:(jb + 1) * 400],
                            in_=psy[:, 0:400])
                    nc.sync.dma_start(
                        out=y.ap()[rt * 128:(rt + 1) * 128, :], in_=ysb)


def make_core_inputs(x, alibi, attention_mask, wq, wk, wv, wo, layer_index):
    li = int(np.asarray(layer_index))
    inv = np.float32(1.0 / (li + 1))
    f32 = np.float32

    xTs = [np.ascontiguousarray(np.asarray(x[b], dtype=f32).T)
           for b in range(B)]

    # packed wo: wo_pk[d, m*1200 + n] = wo.T[m*100+d, n]
    woT = np.asarray(wo, dtype=f32).T                       # [1200, 1200]
    wo_pk = np.ascontiguousarray(
        woT.reshape(NM, D, HID).transpose(1, 0, 2).reshape(D, NM * HID))

    per_group = []
    for g in range(HG):
        sl = slice(g * HL * D, (g + 1) * HL * D)
        # wqk[h, p, c*200 + qk*100 + d] = w{q,k}[g*600 + h*100 + d, c*120 + p]
        wq_g = np.asarray(wq, dtype=f32)[sl, :]             # [600, 1200]
        wk_g = np.asarray(wk, dtype=f32)[sl, :]
        wqk_pk = np.empty((HL, CK, CCH * 2 * D), dtype=f32)
        for h in range(HL):
            qh = wq_g[h * D:(h + 1) * D, :].T               # [1200, 100]
            kh = wk_g[h * D:(h + 1) * D, :].T
            both = np.concatenate(
                [qh.reshape(CCH, CK, D), kh.reshape(CCH, CK, D)],
                axis=2)                                     # [10, 120, 200]
            wqk_pk[h] = both.transpose(1, 0, 2).reshape(CK, CCH * 2 * D)
        # wv_pk[p, c*600 + col] = wv[g*600 + col, c*120 + p]
        wv_g = np.asarray(wv, dtype=f32)[sl, :].T           # [1200, 600]
        wv_pk = np.ascontiguousarray(
            wv_g.reshape(CCH, CK, HL * D).transpose(1, 0, 2)
            .reshape(CK, CCH * HL * D))
        per_group.append((np.ascontiguousarray(wqk_pk), wv_pk))

    in_maps = []
    for b in range(B):
        for g in range(HG):
            a = np.asarray(alibi, dtype=f32)[
                b * H + g * HL: b * H + (g + 1) * HL, 0, :]      # (6, S)
            msk = np.asarray(attention_mask, dtype=f32)[b, 0, 0, :S]
            bias = a * inv + msk[None, :]                        # (6, S)
            biasT = np.ascontiguousarray(
                bias.reshape(HL, TT, 128).transpose(2, 0, 1)
                .reshape(128, HL * TT))
            wqk_pk, wv_pk = per_group[g]
            in_maps.append({
                "xT": xTs[b], "wqk": wqk_pk, "wv": wv_pk,
                "wo": wo_pk, "biasT": biasT,
            })
    scale = float(np.float32(np.sqrt(np.float32(D))) * inv)
    return in_maps, scale


def run(trace=False, **inputs):
    in_maps, scale = make_core_inputs(**inputs)
    nc = build_program(scale)
    res = run_bass_kernel_spmd(nc, in_maps, core_ids=list(range(B * HG)),
                               trace=trace)
    out = np.empty((B, S, HID), dtype=np.float32)
    for b in range(B):
        for g in range(HG):
            out[b, g * ROWS:(g + 1) * ROWS, :] = res.results[b * HG + g]["y"]
    return out, res


def kernel(**inputs) -> np.ndarray:
    out, _ = run(trace=False, **inputs)
    return out


# revision 19
# speedup vs baseline: 1.1587x; 1.1587x over previous
"""Trainium2 Bass kernel for nn_Attention_71966472012100.

Multi-head attention layer (dense_transformer), B=4, S=2048, H=12, D=100,
HID=1200, with the reference's bug-faithful head-mixing reshape before the
output projection.

Sharding: 8 cores = data-parallel over batch (4) x tensor-parallel over head
groups (2 groups of 6 heads). Because the reference reshapes (b, h, s, d) ->
(b, s, h*d) WITHOUT permuting heads back, head-group g's attention output
occupies exactly rows [g*1024, (g+1)*1024) of the reshaped activation.
Each core produces 1024 complete rows of the final output; no cross-core
communication.

v2 (all-f32r): HW measurement shows 16-bit matmuls pay ~240ns per weight
load (stationary changes every matmul here), while f32r weight loads are
free (~215ns for a 512-col matmul = pure streaming). So EVERY matmul in
this kernel is f32r x f32r (also strictly better numerics than the old
bf16 PV/WO path). Key layouts:
  QT/KT: [d, s] per head f32r
  scores^T: [t, s] in PSUM (softmax axis t on partitions; alibi+mask are a
         per-partition bias fused into the ACT exp)
  pt = exp(scores) [t, s] f32r in SBUF (ACT writes f32r directly)
  V': [t, d|ones] f32r (ones column makes PV also produce softmax sums)
  O^T blocks: normalized [d, 512] f32r staged to a DRAM scratch `otd`
         (SBUF cannot hold the full f32 O^T); WO streams it back per
         128-row output block ([d, 1536] chunks).

Schedule (emission order = priority for the dependency-driven scheduler):
  - x^T streams in s-half-major chunk order; V' t-tile groups and the
    first two heads' Q/K s-block projections consume chunks as they land,
    so the PE is busy through the whole x DMA window.
  - attention per head: QK -> exp(ACT, f32r out) -> PV into per-512 PSUM;
    evacuation stages O^T blocks + sums row, deferred-normalizes (recip +
    rank-1 ones broadcast + in-place mul) and DMAs blocks to otd.
  - remaining heads' Q/K proj emitted inside the attention loop (2-deep
    rotating qt/kt buffers) and absorbed into the windows' ACT slack.
  - WO emitted last (lowest priority): per 128-row block, reload the
    [d, 1536] otd chunk, accumulate 12 m-chunks x 3 col-pieces in PSUM,
    stream y out. Runs during the tail windows' bubbles + after.
PSUM (8 banks): startup V'-group 6 + transient 2; attention ss 2x2 +
po 2x1 + transient 2; WO reuses the 2 transient banks for psy.
"""

import numpy as np
import ml_dtypes
from contextlib import ExitStack

import concourse.bass as bass
import concourse.tile as tile
from concourse import bacc
from concourse import mybir
from concourse.bass_utils import run_bass_kernel_spmd

F32 = mybir.dt.float32
F32R = mybir.dt.float32r
BF16 = mybir.dt.bfloat16
EXP = mybir.ActivationFunctionType.Exp

B, S, H, D, HID = 4, 2048, 12, 100, 1200
HG = 2                # head groups (tensor parallel)
HL = H // HG          # 6 heads per core
ROWS = S * HL * D // HID   # 1024 output rows per core
CK, CCH = 120, 10     # contraction chunking of HID
TT = S // 128         # 16 key tiles
VW = HL * (D + 1)     # 606: V' row width per t-tile (d cols + ones col per head)
NM = HID // D         # 12 output-projection contraction chunks
RT = ROWS // 128      # 8 output row blocks
RSPAN = 128 * NM      # 1536 O^T columns per output row block


def _mm(nc, out, lhsT, rhs, **kw):
    nc.tensor.matmul(out, lhsT, rhs, **kw)


def _absorb(nc, ap):
    """PE-side observation of a freshly DMA'd tile (absorbs a DMA wait in a
    throwaway 1-column LDWEIGHTS ahead of the real matmuls)."""
    bb = ap.bitcast(BF16)
    nc.tensor.ldweights(bb[:, 0:1])


def build_program(scale: float, debug_taps: bool = False, n_iters: int = 1):
    nc = bacc.Bacc("TRN2", target_bir_lowering=False, debug=False)

    tn = {}
    tn["xT"] = nc.dram_tensor("xT", [HID, S], F32R, kind="ExternalInput")
    # per-head packed q||k weights: [120, c(10) * 200] f32r
    tn["wqk"] = nc.dram_tensor("wqk", [HL, CK, CCH * 2 * D], F32R,
                               kind="ExternalInput")
    # packed v weights: [120, c(10) * 600] f32r
    tn["wv"] = nc.dram_tensor("wv", [CK, CCH * HL * D], F32R,
                              kind="ExternalInput")
    # packed wo: [100, m(12) * 1200] bf16
    tn["wo"] = nc.dram_tensor("wo", [D, NM * HID], BF16, kind="ExternalInput")
    tn["biasT"] = nc.dram_tensor("biasT", [128, HL * TT], F32,
                                 kind="ExternalInput")
    tn["y"] = nc.dram_tensor("y", [ROWS, HID], F32, kind="ExternalOutput")

    with tile.TileContext(nc) as tc:
        for _ in range(n_iters):
            _emit_iter(nc, tc, tn, scale)
    nc.compile()
    return nc


def _emit_iter(nc, tc, tn, scale):
    xT, wqk, wv, wo, biasT, y = (
        tn["xT"], tn["wqk"], tn["wv"], tn["wo"], tn["biasT"], tn["y"])

    with ExitStack() as ctx:
        pa = ctx.enter_context(tc.tile_pool(name="pa", bufs=1))
        vp = pa.tile([128, TT * VW], F32R, name="vp")
        ot = pa.tile([D, HL * S], BF16, name="ot")
        bias_sb = pa.tile([128, HL * TT], F32, name="bias_sb")
        ones1 = pa.tile([1, D], F32R, name="ones1")
        pqk = ctx.enter_context(tc.tile_pool(name="pqk", bufs=2))
        # bufs=1 is safe ONLY with unified per-head proj emission order:
        # all of head h's proj reads are emitted before head h+1's weights
        # DMA-consumers, keeping the in-order PE stream cycle-free.
        pwqk = ctx.enter_context(tc.tile_pool(name="pwqk", bufs=1))

        qk_tiles = {}
        w_tiles = {}

        def emit_wqk_dma(h):
            w_sb = pwqk.tile([CK, CCH * 2 * D], F32R, tag="wqk", name="w_sb")
            nc.sync.dma_start(out=w_sb, in_=wqk.ap()[h])
            _absorb(nc, w_sb)
            w_tiles[h] = w_sb

        def emit_proj(h, sbs, last):
            """Q/K projection of head h, s-blocks `sbs`, from resident x^T
            into rotating [D, S] f32r tiles (transient PSUM pool)."""
            w_sb = w_tiles[h]
            if h in qk_tiles:
                qt, kt = qk_tiles[h]
            else:
                qt = pqk.tile([D, S], F32R, tag="qt", name="qt")
                kt = pqk.tile([D, S], F32R, tag="kt", name="kt")
                qk_tiles[h] = (qt, kt)
            for sb in sbs:
                for qk, dest in ((0, qt), (1, kt)):
                    acc = ptr.tile([D, 512], F32, tag="tr", name="acc")
                    for c in range(CCH):
                        _mm(nc, acc[:, :],
                            w_sb[:, c * 2 * D + qk * D: c * 2 * D + (qk + 1) * D],
                            xt[:, c * S + sb * 512: c * S + (sb + 1) * 512],
                            start=(c == 0), stop=(c == CCH - 1))
                    nc.vector.tensor_copy(
                        out=dest[:, sb * 512:(sb + 1) * 512], in_=acc[:, :])
            if last:
                w_tiles.pop(h)

        def emit_vprime(tts):
            """V' for t-tiles `tts` (<=3 at a time), both jh head-halves,
            all f32r. accs: 2*len(tts) banks + 2 transient <= 8."""
            with tc.tile_pool(name=f"psv{tts[0]}", bufs=2 * len(tts),
                              space="PSUM") as psv:
                accs = {(jh, t): psv.tile([128, 3 * D], F32, tag="vacc",
                                          name="vacc")
                        for jh in range(2) for t in tts}
                for c in range(CCH):
                    for jh in range(2):
                        for t in tts:
                            _mm(nc, accs[(jh, t)][:, :],
                                xt[:, c * S + t * 128: c * S + t * 128 + 128],
                                wv_sb[:, c * HL * D + jh * 3 * D:
                                      c * HL * D + (jh + 1) * 3 * D],
                                start=(c == 0), stop=(c == CCH - 1))
                for (jh, t), acc in accs.items():
                    dst = vp[:, t * VW + jh * 3 * (D + 1):
                             t * VW + (jh + 1) * 3 * (D + 1)]
                    dst3 = dst.rearrange("p (h e) -> p h e", e=D + 1)
                    nc.vector.tensor_copy(out=dst3[:, :, 0:D],
                                          in_=acc.rearrange(
                                              "p (h d) -> p h d", d=D))

        pending_norm = []

        def emit_norm_finish():
            """recip + rank-1 broadcast + in-place multiply + DMA-out for a
            previously evacuated (h, sh) block pair. Deferred one s-half
            window so the pb matmuls sit behind the next tt-loop in the
            static PE order (they depend on the slow DVE chain)."""
            if not pending_norm:
                return
            for srow, h, col in pending_norm.pop(0):
                srow1 = pnr.tile([1, 1024], F32R, tag="srow1", name="srow1",
                                 bufs=1)
                nc.sync.dma_start(out=srow1, in_=srow[4:5, :])
                rrow = pnr.tile([1, 1024], F32R, tag="rrow", name="rrow",
                                bufs=1)
                with nc.allow_low_precision(reason="f32r recip of sums"):
                    nc.vector.reciprocal(out=rrow, in_=srow1)
                for sbb in range(2):
                    pb = ptr.tile([D, 512], F32, tag="tr", name="pb")
                    _mm(nc, pb[:, :], ones1[0:1, :],
                        rrow[0:1, sbb * 512:(sbb + 1) * 512],
                        start=True, stop=True)
                    oc = h * S + col + sbb * 512
                    nc.vector.tensor_mul(ot[:, oc:oc + 512],
                                         ot[:, oc:oc + 512], pb[:, :])

        def emit_attention_sh(h, sh, qt, kt):
            s0 = sh * 1024
            pos = [ppo.tile([D + 1, 512], F32, tag="po", name="po")
                   for _ in range(2)]
            for tt in range(TT):
                ss = pss.tile([128, 1024], F32, tag="ss", name="ss")
                for sbb in range(2):
                    _mm(nc, ss[:, sbb * 512:(sbb + 1) * 512],
                        kt[:, tt * 128:(tt + 1) * 128],
                        qt[:, s0 + sbb * 512: s0 + (sbb + 1) * 512],
                        start=True, stop=True)
                pt = ppt.tile([128, 1024], F32R, tag="pt", name="pt")
                nc.scalar.activation(
                    out=pt, in_=ss[:, :], func=EXP,
                    bias=bias_sb[:, h * TT + tt: h * TT + tt + 1],
                    scale=scale)
                for sbb in range(2):
                    _mm(nc, pos[sbb][:, :],
                        vp[:, tt * VW + h * (D + 1):
                           tt * VW + (h + 1) * (D + 1)],
                        pt[:, sbb * 512:(sbb + 1) * 512],
                        start=(tt == 0), stop=(tt == TT - 1))
            # evacuation: sums rows (both blocks into one [5,1024] tile) +
            # bf16 O^T copies free each po bank promptly.
            srow = pnr.tile([5, 1024], F32R, tag="srow", name="srow")
            for sbb in range(2):
                nc.vector.tensor_copy(out=srow[:, sbb * 512:(sbb + 1) * 512],
                                      in_=pos[sbb][96:D + 1, :])
                nc.vector.tensor_copy(
                    out=ot[:, h * S + s0 + sbb * 512:
                           h * S + s0 + (sbb + 1) * 512],
                    in_=pos[sbb][0:D, :])
            pending_norm.append([(srow, h, s0)])

        # ================= emission (priority) order =================
        nc.sync.dma_start(out=bias_sb, in_=biasT.ap())

        # shared 2-bank transient PSUM: proj accs, norm pb, and WO psy.
        # Opened OUTSIDE pxt (PSUM/SBUF pool stacks are independent) so the
        # WO block can allocate its accumulators from the already-open pool
        # instead of waiting for the attention PSUM pools to drain.
        with tc.tile_pool(name="ptr", bufs=2, space="PSUM") as ptr:
            with tc.tile_pool(name="pxt", bufs=1) as pxt:
                xt = pxt.tile([CK, CCH * S], F32R, name="xt")
                with tc.tile_pool(name="pwv", bufs=1) as pwv:
                    wv_sb = pwv.tile([CK, CCH * HL * D], F32R, name="wv_sb")
                    nc.vector.memset(ones1.bitcast(F32), 1.0)
                    nc.vector.tensor_copy(out=ones1, in_=ones1.bitcast(F32))
                    # ones cols pre-set; V cols overwritten by V' copies.
                    nc.vector.memset(vp.bitcast(F32), 1.0)
                    nc.vector.tensor_copy(out=vp, in_=vp.bitcast(F32))

                    # x streams in s-half-major order, with wv's c-chunks
                    # riding along the first half, so the V' c-loops (and
                    # later the first heads' s-block projections) start
                    # consuming after the very first chunk pair lands.
                    def x_chunk(half, c):
                        nc.sync.dma_start(
                            out=xt[:, c * S + half * 1024:
                                   c * S + (half + 1) * 1024],
                            in_=xT.ap()[c * CK:(c + 1) * CK,
                                        half * 1024:(half + 1) * 1024])
                        _absorb(nc, xt[:, c * S + half * 1024:
                                       c * S + half * 1024 + 1024])

                    for c in range(CCH):
                        nc.sync.dma_start(
                            out=wv_sb[:, c * HL * D:(c + 1) * HL * D],
                            in_=wv.ap()[:, c * HL * D:(c + 1) * HL * D])
                        x_chunk(0, c)
                    emit_wqk_dma(0)
                    emit_wqk_dma(1)
                    for c in range(CCH):
                        x_chunk(1, c)

                    # consume x chunks as they arrive
                    emit_vprime(range(0, 3))
                    emit_vprime(range(3, 6))
                    emit_vprime(range(6, 8))
                    emit_proj(0, (0, 1, 2, 3), last=True)
                    emit_vprime(range(8, 11))
                    emit_proj(1, (0, 1, 2, 3), last=True)
                    emit_vprime(range(11, 14))
                    emit_vprime(range(14, 16))

                with tc.tile_pool(name="ppt2", bufs=2) as ppt, \
                     tc.tile_pool(name="pnr2", bufs=2) as pnr, \
                     tc.tile_pool(name="pss", bufs=2, space="PSUM") as pss, \
                     tc.tile_pool(name="ppo", bufs=2, space="PSUM") as ppo:
                    for h in range(6):
                        if h >= 2:
                            emit_wqk_dma(h)
                            emit_proj(h, (0, 1, 2, 3), last=True)
                        qt, kt = qk_tiles.pop(h)
                        for sh in range(2):
                            emit_attention_sh(h, sh, qt, kt)
                            if len(pending_norm) > 1:
                                emit_norm_finish()
                    emit_norm_finish()
                    emit_norm_finish()

            # ==== output projection: emitted last (lowest priority) so it
            # fills the tail windows' bubbles. wop lands in xt's freed SBUF
            # (alloc waits on pxt release = last proj read); psy reuses the
            # open ptr transient banks so nothing waits on attention-PSUM
            # drain. rt-outer order streams y out continuously. ====
            with tc.tile_pool(name="pwo", bufs=1) as pwo, \
                 tc.tile_pool(name="pyb", bufs=3) as pyb:
                wop = pwo.tile([D, NM * HID], BF16, name="wop")
                for pc in range(3):
                    nc.sync.dma_start(
                        out=wop[:, pc * 4 * HID:(pc + 1) * 4 * HID],
                        in_=wo.ap()[:, pc * 4 * HID:(pc + 1) * 4 * HID])
                    _absorb(nc, wop[:, pc * 4 * HID:(pc + 1) * 4 * HID])
                ot_r = ot.rearrange("p (r m) -> p r m", m=NM)
                for rt in range(RT):
                    ysb = pyb.tile([128, HID], F32, tag="ysb", name="ysb")
                    for jb in range(3):
                        psy = ptr.tile([128, 512], F32, tag="tr",
                                       name="psy")
                        for m in range(NM):
                            _mm(nc, psy[:, 0:400],
                                ot_r[:, rt * 128:(rt + 1) * 128, m],
                                wop[:, m * HID + jb * 400:
                                    m * HID + (jb + 1) * 400],
                                start=(m == 0), stop=(m == NM - 1))
                        nc.any.tensor_copy(
                            out=ysb[:, jb * 400:(jb + 1) * 400],
                            in_=psy[:, 0:400])
                    nc.sync.dma_start(
                        out=y.ap()[rt * 128:(rt + 1) * 128, :], in_=ysb)


def make_core_inputs(x, alibi, attention_mask, wq, wk, wv, wo, layer_index):
    li = int(np.asarray(layer_index))
    inv = np.float32(1.0 / (li + 1))
    f32 = np.float32

    xTs = [np.ascontiguousarray(np.asarray(x[b], dtype=f32).T)
           for b in range(B)]

    # packed wo: wo_pk[d, m*1200 + n] = wo.T[m*100+d, n]
    woT = np.asarray(wo, dtype=f32).T                       # [1200, 1200]
    wo_pk = np.ascontiguousarray(
        woT.reshape(NM, D, HID).transpose(1, 0, 2).reshape(D, NM * HID)
    ).astype(ml_dtypes.bfloat16)

    per_group = []
    for g in range(HG):
        sl = slice(g * HL * D, (g + 1) * HL * D)
        # wqk[h, p, c*200 + qk*100 + d] = w{q,k}[g*600 + h*100 + d, c*120 + p]
        wq_g = np.asarray(wq, dtype=f32)[sl, :]             # [600, 1200]
        wk_g = np.asarray(wk, dtype=f32)[sl, :]
        wqk_pk = np.empty((HL, CK, CCH * 2 * D), dtype=f32)
        for h in range(HL):
            qh = wq_g[h * D:(h + 1) * D, :].T               # [1200, 100]
            kh = wk_g[h * D:(h + 1) * D, :].T
            both = np.concatenate(
                [qh.reshape(CCH, CK, D), kh.reshape(CCH, CK, D)],
                axis=2)                                     # [10, 120, 200]
            wqk_pk[h] = both.transpose(1, 0, 2).reshape(CK, CCH * 2 * D)
        # wv_pk[p, c*600 + col] = wv[g*600 + col, c*120 + p]
        wv_g = np.asarray(wv, dtype=f32)[sl, :].T           # [1200, 600]
        wv_pk = np.ascontiguousarray(
            wv_g.reshape(CCH, CK, HL * D).transpose(1, 0, 2)
            .reshape(CK, CCH * HL * D))
        per_group.append((np.ascontiguousarray(wqk_pk), wv_pk))

    in_maps = []
    for b in range(B):
        for g in range(HG):
            a = np.asarray(alibi, dtype=f32)[
                b * H + g * HL: b * H + (g + 1) * HL, 0, :]      # (6, S)
            msk = np.asarray(attention_mask, dtype=f32)[b, 0, 0, :S]
            bias = a * inv + msk[None, :]                        # (6, S)
            biasT = np.ascontiguousarray(
                bias.reshape(HL, TT, 128).transpose(2, 0, 1)
                .reshape(128, HL * TT))
            wqk_pk, wv_pk = per_group[g]
            in_maps.append({
                "xT": xTs[b], "wqk": wqk_pk, "wv": wv_pk,
                "wo": wo_pk, "biasT": biasT,
            })
    scale = float(np.float32(np.sqrt(np.float32(D))) * inv)
    return in_maps, scale


def run(trace=False, **inputs):
    in_maps, scale = make_core_inputs(**inputs)
    nc = build_program(scale)
    res = run_bass_kernel_spmd(nc, in_maps, core_ids=list(range(B * HG)),
                               trace=trace)
    out = np.empty((B, S, HID), dtype=np.float32)
    for b in range(B):
        for g in range(HG):
            out[b, g * ROWS:(g + 1) * ROWS, :] = res.results[b * HG + g]["y"]
    return out, res


def kernel(**inputs) -> np.ndarray:
    out, _ = run(trace=False, **inputs)
    return out
